# revision 1
# baseline (speedup 1.0000x reference)
"""Expert-parallel MoE FFN kernel for 8 Trainium2 NeuronCores.

Problem: x (B=4, E=8, N=1024, D=1024) f32; per-expert 2-layer GELU FFN
  h = gelu(x[:,e] @ w1[e] + b1[e]);  out[:,e] = h @ w2[e] + b2[e]
with w1 (E, D, H=4096), w2 (E, H, D).

Sharding: expert-parallel, one expert per core (E == n_cores == 8). Each
core's work is fully independent — no collectives.

Per-core device kernel (fused, weights resident in SBUF, bf16 matmuls with
fp32 PSUM accumulation):
  - host sends xT_e = x_e.T (D, NTOK) bf16 so no on-device transposes needed
  - mm1: hT (H, TN-chunk) = w1.T @ xT   [lhsT = w1 block, rhs = xT chunk]
  - gelu(tanh approx) + b1 on ScalarE during PSUM->SBUF eviction (bf16)
  - mm2: out (TN-chunk, D) = hT.T @ w2  [lhsT = hT block, rhs = w2 block]
  - + b2 on VectorE during PSUM->SBUF eviction (f32), DMA to DRAM
"""

import numpy as np
import ml_dtypes

B, E, N, D, H = 4, 8, 1024, 1024, 4096
NTOK = B * N            # 4096 tokens per expert
P = 128
TN = 256                # token chunk = mm1 moving free dim
NCH = NTOK // TN        # 16 chunks
KO1 = D // P            # 8 k-subtiles for mm1 (contract over D)
KO2 = H // P            # 32 k-subtiles for mm2 (contract over H) = mm1 m-tiles
MT = TN // P            # 2 token subtiles per chunk
NF2 = 512               # mm2 moving free dim (over D)
ND = D // NF2           # 2

_CACHE: dict = {}


def _build_nc(reps=1):
    """Build the per-core Bass program. reps>1 repeats the (idempotent)
    kernel body for marginal-time benchmarking."""
    import concourse.mybir as mybir
    import concourse.tile as tile
    from concourse import bacc

    bf16 = mybir.dt.bfloat16
    f32 = mybir.dt.float32
    gelu = mybir.ActivationFunctionType.Gelu_apprx_tanh
    alu_add = mybir.AluOpType.add

    nc = bacc.Bacc(None, target_bir_lowering=False, debug=False)

    xT = nc.dram_tensor("xT", [D, NTOK], bf16, kind="ExternalInput")
    w1 = nc.dram_tensor("w1", [D, H], bf16, kind="ExternalInput")
    b1 = nc.dram_tensor("b1", [P, KO2], f32, kind="ExternalInput")
    w2 = nc.dram_tensor("w2", [H, D], bf16, kind="ExternalInput")
    b2 = nc.dram_tensor("b2", [P, D], f32, kind="ExternalInput")
    out = nc.dram_tensor("out", [NTOK, D], f32, kind="ExternalOutput")

    xT_v = xT.rearrange("(ko pi) n -> pi ko n", pi=P)     # (128, 8, 4096)
    w1_v = w1.rearrange("(ko pi) h -> pi ko h", pi=P)     # (128, 8, 4096)
    w2_v = w2.rearrange("(ko pi) d -> pi ko d", pi=P)     # (128, 32, 1024)
    out_v = out.rearrange("(mt pi) d -> pi mt d", pi=P)   # (128, 32, 1024)

    with tile.TileContext(nc) as tc:
        with (
            tc.tile_pool(name="wpool", bufs=1) as wpool,
            tc.tile_pool(name="xpool", bufs=2) as xpool,
            tc.tile_pool(name="hpool", bufs=6) as hpool,
            tc.tile_pool(name="opool", bufs=2) as opool,
            tc.tile_pool(name="phpool", bufs=2, space="PSUM") as phpool,
            tc.tile_pool(name="popool", bufs=1, space="PSUM") as popool,
            tc.tile_pool(name="popool2", bufs=2, space="PSUM") as popool2,
        ):
            # w1 split [ko][group] into 4 even 1024-column groups: coarse
            # enough to avoid DMA-queue contention, fine enough that chunk 0
            # only waits ~2.5MB before its first matmul chain. (Finer splits
            # measured worse — per-DMA queue latency dominates.) w2 split
            # per-ko so mm2(m) waits only on piece m.
            GROUP_COLS = [1024] * (H // 1024)
            GROUP_OFF = [0]
            for gc in GROUP_COLS:
                GROUP_OFF.append(GROUP_OFF[-1] + gc)
            # m index -> (group, column offset within group)
            M_TO_GROUP = {}
            for g, (off, gc) in enumerate(zip(GROUP_OFF, GROUP_COLS)):
                for mo in range(gc // P):
                    M_TO_GROUP[(off // P) + mo] = (g, mo)
            w1_sb = [
                [
                    wpool.tile([P, gc], bf16, name=f"w1_sb{ko}_{g}")
                    for g, gc in enumerate(GROUP_COLS)
                ]
                for ko in range(KO1)
            ]
            w2_sb = [wpool.tile([P, D], bf16, name=f"w2_sb{ko}") for ko in range(KO2)]
            b1_sb = wpool.tile([P, KO2], f32, name="b1_sb")
            b2_sb = wpool.tile([P, D], f32, name="b2_sb")

            def load_x(rep, t, split=1):
                xs = [
                    xpool.tile([P, TN], bf16, tag=f"x{ko}", name=f"x_{rep}_{t}_{ko}")
                    for ko in range(KO1)
                ]
                step = TN // split
                for ko in range(KO1):
                    for s in range(split):
                        nc.sync.dma_start(
                            out=xs[ko][:, s * step : (s + 1) * step],
                            in_=xT_v[:, ko, t * TN + s * step : t * TN + (s + 1) * step],
                        )
                return xs

            # DMA issue order = consumption order: x chunk 0, b1, then w1
            # group by group, with w2 pieces interleaved starting after the
            # first few groups (mm2(m) starts ~LAG m-steps after mm1(m)).
            x_next = load_x(0, 0)
            nc.sync.dma_start(out=b1_sb[:], in_=b1[:])
            n_groups = len(GROUP_COLS)
            w2_next = 0
            for g, (off, gc) in enumerate(zip(GROUP_OFF, GROUP_COLS)):
                for ko in range(KO1):
                    nc.sync.dma_start(
                        out=w1_sb[ko][g][:], in_=w1_v[:, ko, off : off + gc]
                    )
                if g == 0:
                    nc.sync.dma_start(out=b2_sb[:], in_=b2[:])
                # trickle w2 pieces in proportion to w1 progress
                w2_target = (g + 1) * KO2 // n_groups
                while w2_next < w2_target:
                    nc.sync.dma_start(
                        out=w2_sb[w2_next][:], in_=w2_v[:, w2_next, :]
                    )
                    w2_next += 1
            while w2_next < KO2:
                nc.sync.dma_start(out=w2_sb[w2_next][:], in_=w2_v[:, w2_next, :])
                w2_next += 1

            # software pipeline: mm2 lags mm1 by LAG m-steps (across chunk
            # boundaries too) so PE never waits on the ScalarE gelu evict or
            # the previous chunk's PSUM eviction.
            LAG = 2
            pend_q = []  # entries: (h_sb, po, m, rep, t)

            def emit_mm2(h_sb, po, m, rep, t, final):
                for mt in range(MT):
                    for n in range(ND):
                        nc.tensor.matmul(
                            po[mt][:, n * NF2 : (n + 1) * NF2],
                            h_sb[:, mt * P : (mt + 1) * P],
                            w2_sb[m][:, n * NF2 : (n + 1) * NF2],
                            start=(m == 0),
                            stop=(m == KO2 - 1),
                        )
                    if final:
                        # evict this mt's accumulator right away (frees its
                        # PSUM slot before the next chunk's mm2 needs it)
                        o_sb = opool.tile(
                            [P, D], f32, tag=f"o{mt}", name=f"o{mt}_{rep}_{t}"
                        )
                        nc.vector.tensor_tensor(
                            o_sb[:], po[mt][:], b2_sb[:], alu_add
                        )
                        nc.sync.dma_start(
                            out=out_v[:, t * MT + mt, :], in_=o_sb[:]
                        )

            def pump(force=False):
                while pend_q and (force or len(pend_q) > LAG):
                    h_sb, po, m, rep, t = pend_q.pop(0)
                    emit_mm2(h_sb, po, m, rep, t, final=(m == KO2 - 1))

            for rep in range(reps):
              for t in range(NCH):
                x_sb = x_next
                po = [
                    popool.tile([P, D], f32, tag="po0", name=f"po0_{rep}_{t}"),
                    popool2.tile([P, D], f32, tag="po1", name=f"po1_{rep}_{t}"),
                ]
                for m in range(KO2):
                    mg, mo = M_TO_GROUP[m]
                    ph = phpool.tile([P, TN], f32, tag="ph", name=f"ph_{rep}_{t}_{m}")
                    for ko in range(KO1):
                        nc.tensor.matmul(
                            ph[:],
                            w1_sb[ko][mg][:, mo * P : (mo + 1) * P],
                            x_sb[ko][:],
                            start=(ko == 0),
                            stop=(ko == KO1 - 1),
                        )
                    h_sb = hpool.tile([P, TN], bf16, tag="h", name=f"h_{rep}_{t}_{m}")
                    nc.scalar.activation(
                        h_sb[:], ph[:], gelu, bias=b1_sb[:, m : m + 1], scale=1.0
                    )
                    pend_q.append((h_sb, po, m, rep, t))
                    pump()
                    if m == 0 and not (t == NCH - 1 and rep == reps - 1):
                        # prefetch next chunk's x while this chunk computes
                        tn, rn = (t + 1, rep) if t < NCH - 1 else (0, rep + 1)
                        x_next = load_x(rn, tn)
            pump(force=True)

    nc.compile()
    return nc


def _get_nc(reps=1):
    key = f"nc{reps}"
    if key not in _CACHE:
        _CACHE[key] = _build_nc(reps)
    return _CACHE[key]


def _make_runner(nc):
    """Build a jitted SPMD executor for an arbitrary finalized Bass module.

    Mirrors concourse.bass2jax.run_bass_via_pjrt's multi-core branch, but
    returns a reusable jitted function (no re-trace/re-compile per call).
    """
    import jax
    from jax.experimental.shard_map import shard_map
    from jax.sharding import Mesh, NamedSharding, PartitionSpec

    import concourse.mybir as mybir
    from concourse import bass2jax

    bass2jax.install_neuronx_cc_hook()

    partition_name = (
        nc.partition_id_tensor.name if nc.partition_id_tensor else None
    )
    in_names = []
    out_names = []
    out_avals = []
    zero_out_specs = []
    for alloc in nc.m.functions[0].allocations:
        if not isinstance(alloc, mybir.MemoryLocationSet):
            continue
        name = alloc.memorylocations[0].name
        if alloc.kind == "ExternalInput":
            if name != partition_name:
                in_names.append(name)
        elif alloc.kind == "ExternalOutput":
            shape = tuple(alloc.tensor_shape)
            dtype = mybir.dt.np(alloc.dtype)
            out_names.append(name)
            out_avals.append(jax.core.ShapedArray(shape, dtype))
            zero_out_specs.append((shape, dtype))
    n_params = len(in_names)
    n_outs = len(out_names)
    all_in_names = list(in_names) + list(out_names)
    if partition_name is not None:
        all_in_names.append(partition_name)
    donate = tuple(range(n_params, n_params + n_outs))

    def _body(*args):
        operands = list(args)
        if partition_name is not None:
            operands.append(bass2jax.partition_id_tensor())
        outs = bass2jax._bass_exec_p.bind(
            *operands,
            out_avals=tuple(out_avals),
            in_names=tuple(all_in_names),
            out_names=tuple(out_names),
            lowering_input_output_aliases=(),
            sim_require_finite=True,
            sim_require_nnan=True,
            nc=nc,
        )
        return tuple(outs)

    devices = jax.devices()[:E]
    mesh = Mesh(np.asarray(devices), ("core",))
    in_specs = (PartitionSpec("core"),) * (n_params + n_outs)
    out_specs = (PartitionSpec("core"),) * n_outs
    fn = jax.jit(
        shard_map(
            _body, mesh=mesh, in_specs=in_specs, out_specs=out_specs,
            check_rep=False,
        ),
        donate_argnums=donate,
        keep_unused=True,
    )
    sharding = NamedSharding(mesh, PartitionSpec("core"))
    return {
        "fn": fn,
        "in_names": in_names,
        "out_names": out_names,
        "out_avals": out_avals,
        "zero_out_specs": zero_out_specs,
        "sharding": sharding,
    }


def _get_runner(reps=1):
    key = f"runner{reps}"
    if key not in _CACHE:
        _CACHE[key] = _make_runner(_get_nc(reps))
    return _CACHE[key]


def _exec_spmd(in_maps, reps=1):
    """Run the cached executor on per-core input dicts; returns per-core
    output dicts."""
    import jax

    r = _get_runner(reps)
    concat_in = [
        np.concatenate([np.asarray(m[name]) for m in in_maps], axis=0)
        for name in r["in_names"]
    ]

    def _call():
        zeros = [
            np.zeros((E * shape[0], *shape[1:]), dtype)
            for shape, dtype in r["zero_out_specs"]
        ]
        outs = r["fn"](*concat_in, *zeros)
        for o in outs:
            o.block_until_ready()
        return outs

    try:
        out_arrs = _call()
    except Exception:
        # transient device errors (e.g. NRT exec-unit unrecoverable) have
        # been observed to clear on retry
        import time as _time

        _time.sleep(5.0)
        out_arrs = _call()
    results = []
    for c in range(E):
        results.append(
            {
                name: np.asarray(out_arrs[i]).reshape(
                    E, *r["out_avals"][i].shape
                )[c]
                for i, name in enumerate(r["out_names"])
            }
        )
    return results


def _prepare_in_maps(x, w1, b1, w2, b2):
    concat = _prepare_concat(x, w1, b1, w2, b2)
    return [
        {name: arr.reshape(E, -1, *arr.shape[1:])[e] for name, arr in concat.items()}
        for e in range(E)
    ]


def _prepare_concat(x, w1, b1, w2, b2):
    """Vectorized host prep: build the per-core inputs already concatenated
    along axis 0 (the layout the sharded executor wants) in one transform
    per tensor instead of 16 per-expert copies."""
    bf16 = ml_dtypes.bfloat16
    x = np.asarray(x)
    # xT concat: row block e is x_e.T (D, NTOK); xT[e][d, b*N+n] = x[b,e,n,d]
    # (cast happens in the same pass as the transpose copy; parallel over
    # experts — np.copyto releases the GIL for the bulk of the work)
    from concurrent.futures import ThreadPoolExecutor

    xT_c = np.empty((E, D, NTOK), dtype=bf16)
    xt_view = np.transpose(x, (1, 3, 0, 2)).reshape(E, D, NTOK)

    def _cast_expert(e):
        np.copyto(xT_c[e], xt_view[e], casting="unsafe")

    with ThreadPoolExecutor(max_workers=8) as pool:
        list(pool.map(_cast_expert, range(E)))
    xT_c = xT_c.reshape(E * D, NTOK)
    w1_c = np.asarray(w1, dtype=bf16).reshape(E * D, H)
    w2_c = np.asarray(w2, dtype=bf16).reshape(E * H, D)
    # b1 per-expert (H,) -> (P, KO2) partition-major view
    b1_c = np.ascontiguousarray(
        np.asarray(b1, dtype=np.float32).reshape(E, KO2, P).transpose(0, 2, 1)
    ).reshape(E * P, KO2)
    b2_c = np.ascontiguousarray(
        np.broadcast_to(
            np.asarray(b2, dtype=np.float32)[:, None, :], (E, P, D)
        )
    ).reshape(E * P, D)
    return {"xT": xT_c, "w1": w1_c, "b1": b1_c, "w2": w2_c, "b2": b2_c}


def _exec_concat(concat, reps=1):
    """Run the cached executor on pre-concatenated inputs; returns the raw
    concatenated output arrays keyed by name."""
    import jax.numpy as jnp

    r = _get_runner(reps)
    concat_in = [concat[name] for name in r["in_names"]]

    def _call():
        # create donated output buffers on-device (a 134MB host->device
        # transfer of zeros otherwise dominates the call)
        zeros = [
            jnp.zeros((E * shape[0], *shape[1:]), dtype, device=r["sharding"])
            for shape, dtype in r["zero_out_specs"]
        ]
        outs = r["fn"](*concat_in, *zeros)
        for o in outs:
            o.block_until_ready()
        return outs

    try:
        out_arrs = _call()
    except Exception:
        # transient device errors (e.g. NRT exec-unit unrecoverable) have
        # been observed to clear on retry
        import time as _time

        _time.sleep(5.0)
        out_arrs = _call()
    return {name: np.asarray(out_arrs[i]) for i, name in enumerate(r["out_names"])}


def _run(x, w1, b1, w2, b2):
    concat = _prepare_concat(x, w1, b1, w2, b2)
    res = _exec_concat(concat)
    # out rows: block e = (NTOK, D) with token index b*N+n
    return np.ascontiguousarray(
        res["out"].reshape(E, B, N, D).transpose(1, 0, 2, 3)
    )


def kernel(x, w1, b1, w2, b2):
    return _run(x, w1, b1, w2, b2)



# revision 2
# speedup vs baseline: 1503.8142x; 1503.8142x over previous
"""Expert-parallel MoE FFN kernel for 8 Trainium2 NeuronCores (v3).

Problem: x (B=4, E=8, N=1024, D=1024) f32; per-expert 2-layer GELU FFN
  h = gelu(x[:,e] @ w1[e] + b1[e]);  out[:,e] = h @ w2[e] + b2[e]
with w1 (E, D, H=4096), w2 (E, H, D).

Sharding: expert-parallel, one expert per core (E == n_cores == 8); no
collectives.

v3 = v2 + partial-fp8 mm2: the last MF8=6 of 32 H-blocks of mm2's
contraction run as 3 fp8e4 DoubleRow pairs (2 MACs/cell/cycle) instead
of bf16 — gelu evicts those blocks straight to fp8, w2 rows for them are
pre-quantized on the host. Error budget (validated vs CPU sim at full
size): bf16-only 0.34%, with 768 fp8 rows 1.66% — under the 2e-2 gate
with margin. w2/b2 carry an exact x16 power-of-two scale so the fp8 w2
sits mid-range (unscaled it straddles e4m3's min-normal and quantizes as
subnormals); the f32 output is descaled by 1/16 on the host, also exact.
"""

import numpy as np
import ml_dtypes

B, E, N, D, H = 4, 8, 1024, 1024, 4096
NTOK = B * N            # 4096 tokens per expert
P = 128
TNP = 512               # mm1 pair-chunk tokens = mm1 moving free dim
NPCH = NTOK // TNP      # 8 pair-chunks
TN = 256                # mm2 sub-chunk tokens
KO1 = D // P            # 8 k-subtiles for mm1 (contract over D)
KO2 = H // P            # 32 k-subtiles for mm2 (contract over H) = mm1 m-tiles
MT = TN // P            # 2 token subtiles per mm2 sub-chunk
NF2 = 512               # mm2 moving free dim (over D)
ND = D // NF2           # 2
MF8 = 6                 # trailing mm2 H-blocks computed in fp8 (DoubleRow)
MBF = KO2 - MF8         # leading bf16 H-blocks
NQ = MF8 // 2           # fp8 DoubleRow pairs
SW = 16.0               # exact pow2 scale on w2/b2 (descale on host)

_CACHE: dict = {}


def _build_nc(reps=1):
    """Build the per-core Bass program. reps>1 repeats the (idempotent)
    kernel body for marginal-time benchmarking."""
    import concourse.mybir as mybir
    import concourse.tile as tile
    from concourse import bacc

    bf16 = mybir.dt.bfloat16
    f32 = mybir.dt.float32
    f8 = mybir.dt.float8e4
    gelu = mybir.ActivationFunctionType.Gelu_apprx_tanh
    alu_add = mybir.AluOpType.add
    DR = mybir.MatmulPerfMode.DoubleRow

    nc = bacc.Bacc(None, target_bir_lowering=False, debug=False)

    xT = nc.dram_tensor("xT", [D, NTOK], bf16, kind="ExternalInput")
    w1 = nc.dram_tensor("w1", [D, H], bf16, kind="ExternalInput")
    b1 = nc.dram_tensor("b1", [P, KO2], f32, kind="ExternalInput")
    w2 = nc.dram_tensor("w2", [H, D], bf16, kind="ExternalInput")
    w2f8 = nc.dram_tensor("w2f8", [MF8 * P, D], f8, kind="ExternalInput")
    b2 = nc.dram_tensor("b2", [P, D], f32, kind="ExternalInput")
    out = nc.dram_tensor("out", [NTOK, D], f32, kind="ExternalOutput")

    xT_v = xT.rearrange("(ko pi) n -> pi ko n", pi=P)     # (128, 8, 4096)
    w1_v = w1.rearrange("(ko pi) h -> pi ko h", pi=P)     # (128, 8, 4096)
    w2_v = w2.rearrange("(ko pi) d -> pi ko d", pi=P)     # (128, 32, 1024)
    w2f8_v = w2f8.rearrange("(ko pi) d -> pi ko d", pi=P)  # (128, 6, 1024)
    out_v = out.rearrange("(mt pi) d -> pi mt d", pi=P)   # (128, 32, 1024)

    with tile.TileContext(nc) as tc:
        with (
            tc.tile_pool(name="wpool", bufs=1) as wpool,
            tc.tile_pool(name="xpool", bufs=2) as xpool,
            tc.tile_pool(name="hpool", bufs=1) as hpool,
            tc.tile_pool(name="opool", bufs=2) as opool,
            tc.tile_pool(name="phpool", bufs=2, space="PSUM") as phpool,
            tc.tile_pool(name="popool", bufs=1, space="PSUM") as popool,
            tc.tile_pool(name="popool2", bufs=2, space="PSUM") as popool2,
        ):
            # w1 split [ko][group] into 4 even 1024-column groups: coarse
            # enough to avoid DMA-queue contention, fine enough that chunk 0
            # only waits ~2.5MB before its first matmul chain. w2 split
            # per-ko so mm2(m) waits only on piece m.
            GROUP_COLS = [1024] * (H // 1024)
            GROUP_OFF = [0]
            for gc in GROUP_COLS:
                GROUP_OFF.append(GROUP_OFF[-1] + gc)
            # m index -> (group, column offset within group)
            M_TO_GROUP = {}
            for g, (off, gc) in enumerate(zip(GROUP_OFF, GROUP_COLS)):
                for mo in range(gc // P):
                    M_TO_GROUP[(off // P) + mo] = (g, mo)
            w1_sb = [
                [
                    wpool.tile([P, gc], bf16, name=f"w1_sb{ko}_{g}")
                    for g, gc in enumerate(GROUP_COLS)
                ]
                for ko in range(KO1)
            ]
            w2_sb = [wpool.tile([P, D], bf16, name=f"w2_sb{ko}") for ko in range(MBF)]
            w2f8_sb = [
                wpool.tile([P, 2, D], f8, name=f"w2f8_sb{q}") for q in range(NQ)
            ]
            b1_sb = wpool.tile([P, KO2], f32, name="b1_sb")
            b2_sb = wpool.tile([P, D], f32, name="b2_sb")

            def load_x(rep, p):
                xs = [
                    xpool.tile([P, TNP], bf16, tag=f"x{ko}", name=f"x_{rep}_{p}_{ko}")
                    for ko in range(KO1)
                ]
                for ko in range(KO1):
                    nc.sync.dma_start(
                        out=xs[ko][:],
                        in_=xT_v[:, ko, p * TNP : (p + 1) * TNP],
                    )
                return xs

            def load_w2_piece(m):
                """DMA the weights mm2 consumes at logical m-block m."""
                if m < MBF:
                    nc.sync.dma_start(out=w2_sb[m][:], in_=w2_v[:, m, :])
                else:
                    i = m - MBF
                    nc.sync.dma_start(
                        out=w2f8_sb[i // 2][:, i % 2, :],
                        in_=w2f8_v[:, i, :],
                    )

            # DMA issue order = consumption order: x chunk 0, b1, then w1
            # group by group, with w2 pieces interleaved starting after the
            # first few groups (mm2(m) starts ~LAG m-steps after mm1(m)).
            x_next = load_x(0, 0)
            nc.sync.dma_start(out=b1_sb[:], in_=b1[:])
            n_groups = len(GROUP_COLS)
            w2_next = 0
            for g, (off, gc) in enumerate(zip(GROUP_OFF, GROUP_COLS)):
                for ko in range(KO1):
                    nc.sync.dma_start(
                        out=w1_sb[ko][g][:], in_=w1_v[:, ko, off : off + gc]
                    )
                if g == 0:
                    nc.sync.dma_start(out=b2_sb[:], in_=b2[:])
                # trickle w2 pieces in proportion to w1 progress
                w2_target = (g + 1) * KO2 // n_groups
                while w2_next < w2_target:
                    load_w2_piece(w2_next)
                    w2_next += 1
            while w2_next < KO2:
                load_w2_piece(w2_next)
                w2_next += 1

            def emit_mm2_step(item, po, sub, mt, rep, p):
                """One mm2 item for (sub-chunk, mt): ('bf', m) is 2 bf16 MMs
                over D-halves; ('f8', q) is 2 DoubleRow MMs covering H-blocks
                MBF+2q and MBF+2q+1. Evict po[mt] after the last item."""
                kind, idx, h_sb = item
                t = 2 * p + sub  # global 256-token chunk index
                col0 = sub * TN + mt * P
                for n in range(ND):
                    if kind == "bf":
                        nc.tensor.matmul(
                            po[mt][:, n * NF2 : (n + 1) * NF2],
                            h_sb[:, col0 : col0 + P],
                            w2_sb[idx][:, n * NF2 : (n + 1) * NF2],
                            start=(idx == 0),
                            stop=False,
                        )
                    else:
                        nc.tensor.matmul(
                            po[mt][:, n * NF2 : (n + 1) * NF2],
                            h_sb[:, :, col0 : col0 + P],
                            w2f8_sb[idx][:, :, n * NF2 : (n + 1) * NF2],
                            start=False,
                            stop=(idx == NQ - 1),
                            perf_mode=DR,
                        )
                if kind == "f8" and idx == NQ - 1:
                    o_sb = opool.tile(
                        [P, D], f32, tag=f"o{mt}", name=f"o{mt}_{rep}_{t}"
                    )
                    nc.vector.tensor_tensor(
                        o_sb[:], po[mt][:], b2_sb[:], alu_add
                    )
                    nc.sync.dma_start(
                        out=out_v[:, t * MT + mt, :], in_=o_sb[:]
                    )

            # software pipeline: sub-chunk A's mm2 lags mm1 by LAG m-steps so
            # PE never waits on the ScalarE gelu evict.
            LAG = 2

            for rep in range(reps):
              for p in range(NPCH):
                x_sb = x_next
                poA = [
                    popool.tile([P, D], f32, tag="po0", name=f"poA0_{rep}_{p}"),
                    popool2.tile([P, D], f32, tag="po1", name=f"poA1_{rep}_{p}"),
                ]
                items = []   # mm2 work items in accumulation order
                pend = []
                hf8_tiles = {}
                for m in range(KO2):
                    mg, mo = M_TO_GROUP[m]
                    ph = phpool.tile([P, TNP], f32, tag="ph", name=f"ph_{rep}_{p}_{m}")
                    for ko in range(KO1):
                        nc.tensor.matmul(
                            ph[:],
                            w1_sb[ko][mg][:, mo * P : (mo + 1) * P],
                            x_sb[ko][:],
                            start=(ko == 0),
                            stop=(ko == KO1 - 1),
                        )
                    if m < MBF:
                        h_sb = hpool.tile(
                            [P, TNP], bf16, tag=f"h{m}", name=f"h_{rep}_{p}_{m}"
                        )
                        nc.scalar.activation(
                            h_sb[:], ph[:], gelu, bias=b1_sb[:, m : m + 1], scale=1.0
                        )
                        pend.append(("bf", m, h_sb))
                    else:
                        i = m - MBF
                        q, half = i // 2, i % 2
                        if half == 0:
                            hf8 = hpool.tile(
                                [P, 2, TNP], f8, tag=f"hf8{q}",
                                name=f"hf8_{rep}_{p}_{q}",
                            )
                            hf8_tiles[q] = hf8
                        else:
                            hf8 = hf8_tiles[q]
                        nc.scalar.activation(
                            hf8[:, half, :], ph[:], gelu,
                            bias=b1_sb[:, m : m + 1], scale=1.0,
                        )
                        if half == 1:
                            pend.append(("f8", q, hf8))
                    while len(pend) > LAG:
                        it = pend.pop(0)
                        items.append(it)
                        for mt in range(MT):
                            emit_mm2_step(it, poA, 0, mt, rep, p)
                    if m == 0 and not (p == NPCH - 1 and rep == reps - 1):
                        # prefetch next pair-chunk's x while this one computes
                        pn, rn = (p + 1, rep) if p < NPCH - 1 else (0, rep + 1)
                        x_next = load_x(rn, pn)
                while pend:
                    it = pend.pop(0)
                    items.append(it)
                    for mt in range(MT):
                        emit_mm2_step(it, poA, 0, mt, rep, p)
                # sub-chunk B sweep: po1 (double-buffered) leads; po0
                # (single-buffered) lags B_STAG items so its first MM lands
                # after sub-chunk A's po0 eviction has drained.
                poB = [
                    popool.tile([P, D], f32, tag="po0", name=f"poB0_{rep}_{p}"),
                    popool2.tile([P, D], f32, tag="po1", name=f"poB1_{rep}_{p}"),
                ]
                B_STAG = 3
                n_items = len(items)
                for j in range(n_items + B_STAG):
                    if j < n_items:
                        emit_mm2_step(items[j], poB, 1, 1, rep, p)
                    if j >= B_STAG:
                        emit_mm2_step(items[j - B_STAG], poB, 1, 0, rep, p)

    nc.compile()
    return nc


def _get_nc(reps=1):
    key = f"nc{reps}"
    if key not in _CACHE:
        _CACHE[key] = _build_nc(reps)
    return _CACHE[key]


def _make_runner(nc):
    """Build a jitted SPMD executor for an arbitrary finalized Bass module.

    Mirrors concourse.bass2jax.run_bass_via_pjrt's multi-core branch, but
    returns a reusable jitted function (no re-trace/re-compile per call).
    """
    import jax
    from jax.experimental.shard_map import shard_map
    from jax.sharding import Mesh, NamedSharding, PartitionSpec

    import concourse.mybir as mybir
    from concourse import bass2jax

    bass2jax.install_neuronx_cc_hook()

    partition_name = (
        nc.partition_id_tensor.name if nc.partition_id_tensor else None
    )
    in_names = []
    in_specs_list = []
    out_names = []
    out_avals = []
    zero_out_specs = []
    for alloc in nc.m.functions[0].allocations:
        if not isinstance(alloc, mybir.MemoryLocationSet):
            continue
        name = alloc.memorylocations[0].name
        if alloc.kind == "ExternalInput":
            if name != partition_name:
                in_names.append(name)
                in_specs_list.append(
                    (name, tuple(alloc.tensor_shape), mybir.dt.np(alloc.dtype))
                )
        elif alloc.kind == "ExternalOutput":
            shape = tuple(alloc.tensor_shape)
            dtype = mybir.dt.np(alloc.dtype)
            out_names.append(name)
            out_avals.append(jax.core.ShapedArray(shape, dtype))
            zero_out_specs.append((shape, dtype))
    n_params = len(in_names)
    n_outs = len(out_names)
    all_in_names = list(in_names) + list(out_names)
    if partition_name is not None:
        all_in_names.append(partition_name)
    donate = tuple(range(n_params, n_params + n_outs))

    def _body(*args):
        operands = list(args)
        if partition_name is not None:
            operands.append(bass2jax.partition_id_tensor())
        outs = bass2jax._bass_exec_p.bind(
            *operands,
            out_avals=tuple(out_avals),
            in_names=tuple(all_in_names),
            out_names=tuple(out_names),
            lowering_input_output_aliases=(),
            sim_require_finite=True,
            sim_require_nnan=True,
            nc=nc,
        )
        return tuple(outs)

    devices = jax.devices()[:E]
    mesh = Mesh(np.asarray(devices), ("core",))
    in_specs = (PartitionSpec("core"),) * (n_params + n_outs)
    out_specs = (PartitionSpec("core"),) * n_outs
    fn = jax.jit(
        shard_map(
            _body, mesh=mesh, in_specs=in_specs, out_specs=out_specs,
            check_rep=False,
        ),
        donate_argnums=donate,
        keep_unused=True,
    )
    sharding = NamedSharding(mesh, PartitionSpec("core"))
    return {
        "fn": fn,
        "in_names": in_names,
        "in_specs": in_specs_list,
        "out_names": out_names,
        "out_avals": out_avals,
        "zero_out_specs": zero_out_specs,
        "sharding": sharding,
    }


def _get_runner(reps=1):
    key = f"runner{reps}"
    if key not in _CACHE:
        _CACHE[key] = _make_runner(_get_nc(reps))
    return _CACHE[key]


def _prepare_concat(x, w1, b1, w2, b2):
    """Vectorized host prep: build the per-core inputs already concatenated
    along axis 0 (the layout the sharded executor wants) in one transform
    per tensor instead of 16 per-expert copies."""
    bf16 = ml_dtypes.bfloat16
    f8 = ml_dtypes.float8_e4m3
    x = np.asarray(x)
    # xT concat: row block e is x_e.T (D, NTOK); xT[e][d, b*N+n] = x[b,e,n,d]
    # (cast happens in the same pass as the transpose copy; parallel over
    # experts — np.copyto releases the GIL for the bulk of the work)
    from concurrent.futures import ThreadPoolExecutor

    xT_c = np.empty((E, D, NTOK), dtype=bf16)
    xt_view = np.transpose(x, (1, 3, 0, 2)).reshape(E, D, NTOK)

    def _cast_expert(e):
        np.copyto(xT_c[e], xt_view[e], casting="unsafe")

    with ThreadPoolExecutor(max_workers=8) as pool:
        list(pool.map(_cast_expert, range(E)))
    xT_c = xT_c.reshape(E * D, NTOK)
    w1_c = np.asarray(w1, dtype=bf16).reshape(E * D, H)
    w2_np = np.asarray(w2, dtype=np.float32)
    # mm2 runs at an exact x16 scale (descale on host): bf16 blocks and b2
    # carry x16; the trailing MF8 H-blocks are quantized to fp8e4 at x16 so
    # their values sit mid-range instead of straddling e4m3's min-normal.
    w2_c = (w2_np * np.float32(SW)).astype(bf16).reshape(E * H, D)
    w2f8_c = np.ascontiguousarray(
        (w2_np[:, MBF * P :, :] * np.float32(SW))
    ).astype(f8).reshape(E * MF8 * P, D)
    # b1 per-expert (H,) -> (P, KO2) partition-major view
    b1_c = np.ascontiguousarray(
        np.asarray(b1, dtype=np.float32).reshape(E, KO2, P).transpose(0, 2, 1)
    ).reshape(E * P, KO2)
    b2_c = np.ascontiguousarray(
        np.broadcast_to(
            np.asarray(b2, dtype=np.float32)[:, None, :] * np.float32(SW),
            (E, P, D),
        )
    ).reshape(E * P, D)
    return {
        "xT": xT_c, "w1": w1_c, "b1": b1_c,
        "w2": w2_c, "w2f8": w2f8_c, "b2": b2_c,
    }


def _exec_concat(concat, reps=1):
    """Run the cached executor on pre-concatenated inputs; returns the raw
    concatenated output arrays keyed by name."""
    import jax.numpy as jnp

    r = _get_runner(reps)
    concat_in = [concat[name] for name in r["in_names"]]

    def _call():
        # create donated output buffers on-device (a 134MB host->device
        # transfer of zeros otherwise dominates the call)
        zeros = [
            jnp.zeros((E * shape[0], *shape[1:]), dtype, device=r["sharding"])
            for shape, dtype in r["zero_out_specs"]
        ]
        outs = r["fn"](*concat_in, *zeros)
        for o in outs:
            o.block_until_ready()
        return outs

    try:
        out_arrs = _call()
    except Exception:
        # transient device errors (e.g. NRT exec-unit unrecoverable) have
        # been observed to clear on retry
        import time as _time

        _time.sleep(5.0)
        out_arrs = _call()
    return {name: np.asarray(out_arrs[i]) for i, name in enumerate(r["out_names"])}


def _run(x, w1, b1, w2, b2):
    concat = _prepare_concat(x, w1, b1, w2, b2)
    res = _exec_concat(concat)
    # out rows: block e = (NTOK, D) with token index b*N+n; device output
    # carries the exact x16 mm2 scale — divide out here (exact pow2)
    return np.ascontiguousarray(
        res["out"].reshape(E, B, N, D).transpose(1, 0, 2, 3)
        * np.float32(1.0 / SW)
    )


def kernel(x, w1, b1, w2, b2):
    return _run(x, w1, b1, w2, b2)
